# revision 9
# baseline (speedup 1.0000x reference)
"""DIN (deep interest network) forward pass on 8 Trainium2 NeuronCores.

Strategy: pure data-parallel over the batch (1024 rows/core). The large
history tensor is staged host-side into two bf16 layouts -- [F, B, S]
(feature-major, feeds the attention-score matmuls) and [S, B, F]
(seq-major, feeds the attention/mean pooling matmuls) -- so the device
never transposes the big tensor.  Total HBM traffic per core matches a
single fp32 load of the original tensor (~105 MB).

All matmuls contract on the partition dim with fp32 PSUM accumulation;
weights are pre-folded host-side (Wi@Wk, Wi@Wq, batchnorm scale/shift).
"""
import sys
import numpy as np

sys.path.insert(0, "/opt/trn_rl_repo")

import concourse.bass as bass
import concourse.bacc as bacc
import concourse.tile as tile
from concourse import mybir
from concourse import bass_utils
import ml_dtypes

BF16NP = ml_dtypes.bfloat16
F32 = mybir.dt.float32
BF = mybir.dt.bfloat16
AF = mybir.ActivationFunctionType
ALU = mybir.AluOpType

NCORES = 8
B, S, F, E, A = 8192, 200, 128, 64, 64
NB = B // NCORES          # batch rows per core
EPS = 1e-5


# ---------------------------------------------------------------- device ----

def _declare_inputs(nc, nb):
    d = {}

    def din(name, shape, dt):
        d[name] = nc.dram_tensor(name, shape, dt, kind="ExternalInput").ap()

    din("histT", [F, nb, S], BF)        # feature-major history
    din("nat", [S, nb, F], BF)          # seq-major history
    din("maskadd", [nb, S], F32)        # 0 / -1e9 additive score mask
    din("mlT", [S, nb], BF)             # (s<len)/len pooling weights, seq-major
    din("candT_eo", [F, nb], BF)        # cand^T, even b in first half, odd in second
    din("candT_asc", [F, nb], BF)       # cand^T, ascending b
    din("wik2a", [F, 128], BF)          # [Wi@Wk | 0]
    din("wik2b", [F, 128], BF)          # [0 | Wi@Wk]
    din("wiq2a", [F, 128], BF)          # [Wi@Wq | 0]
    din("wiq2b", [F, 128], BF)          # [0 | Wi@Wq]
    din("wi", [F, E], BF)
    din("wv2", [2 * A, 2], BF)          # blockdiag([Wv],[Wv])
    din("c0_2", [2 * A, 1], F32)        # [bi@Wk + bi@Wq] stacked twice
    din("bi2", [E, 1], F32)
    din("identf", [128, 128], F32)
    din("identb", [128, 128], BF)
    din("w1a", [64, 128], BF)
    din("w1b", [64, 128], BF)
    din("w1c", [64, 128], BF)
    din("bn1s", [128, 1], F32)
    din("bn1h", [128, 1], F32)
    din("w2", [128, 64], BF)
    din("bn2s", [64, 1], F32)
    din("bn2h", [64, 1], F32)
    din("w3", [64, 32], BF)
    din("bn3s", [32, 1], F32)
    din("bn3h", [32, 1], F32)
    din("wo", [32, 1], BF)
    din("bo2", [1, 1], F32)
    return d


def _body(tc, d, out_d, nb):
    nc = tc.nc
    import contextlib
    ctx = contextlib.ExitStack()
    S0, S1 = 128, S - 128               # seq chunks on partitions
    G = nb // 128                       # 128-batch groups
    NPAIR = nb // 2
    with ctx:
        consts = ctx.enter_context(tc.tile_pool(name="consts", bufs=1))
        persist = ctx.enter_context(tc.tile_pool(name="persist", bufs=1))
        histT_p = ctx.enter_context(tc.tile_pool(name="histT", bufs=3))
        hid_p = ctx.enter_context(tc.tile_pool(name="hid", bufs=4))
        grp_p = ctx.enter_context(tc.tile_pool(name="grp", bufs=2))
        nat_p = ctx.enter_context(tc.tile_pool(name="nat", bufs=3))
        mlp_p = ctx.enter_context(tc.tile_pool(name="mlp", bufs=2))

        def load_const(name, shape, dt):
            t = consts.tile(shape, dt, tag=name)
            nc.sync.dma_start(out=t, in_=d[name])
            return t

        wik2a = load_const("wik2a", [F, 128], BF)
        wik2b = load_const("wik2b", [F, 128], BF)
        wiq2a = load_const("wiq2a", [F, 128], BF)
        wiq2b = load_const("wiq2b", [F, 128], BF)
        wi = load_const("wi", [F, E], BF)
        wv2 = load_const("wv2", [2 * A, 2], BF)
        c0_2 = load_const("c0_2", [2 * A, 1], F32)
        bi2 = load_const("bi2", [E, 1], F32)
        identf = load_const("identf", [128, 128], F32)
        identb = load_const("identb", [128, 128], BF)
        w1a = load_const("w1a", [64, 128], BF)
        w1b = load_const("w1b", [64, 128], BF)
        w1c = load_const("w1c", [64, 128], BF)
        bn1s = load_const("bn1s", [128, 1], F32)
        bn1h = load_const("bn1h", [128, 1], F32)
        w2 = load_const("w2", [128, 64], BF)
        bn2s = load_const("bn2s", [64, 1], F32)
        bn2h = load_const("bn2h", [64, 1], F32)
        w3 = load_const("w3", [64, 32], BF)
        bn3s = load_const("bn3s", [32, 1], F32)
        bn3h = load_const("bn3h", [32, 1], F32)
        wo = load_const("wo", [32, 1], BF)
        bo2 = load_const("bo2", [1, 1], F32)
        candT_eo = load_const("candT_eo", [F, nb], BF)
        candT_asc = load_const("candT_asc", [F, nb], BF)
        mlT0 = consts.tile([S0, nb], BF, tag="mlT0")
        nc.sync.dma_start(out=mlT0, in_=d["mlT"][0:S0, :])
        mlT1 = consts.tile([S1, nb], BF, tag="mlT1")
        nc.sync.dma_start(out=mlT1, in_=d["mlT"][S0:S, :])

        # persistent buffers
        qh2 = persist.tile([2 * A, NPAIR], F32, tag="qh2")
        cembT = persist.tile([E, nb], BF, tag="cembT")
        poolW0 = persist.tile([S0, nb, 2], BF, tag="poolW0")
        poolW1 = persist.tile([S1, nb, 2], BF, tag="poolW1")
        irT = persist.tile([F, nb], BF, tag="irT")
        avT = persist.tile([F, nb], BF, tag="avT")

        NCH = min(512, nb)
        with tc.tile_pool(name="pro_ps", bufs=2, space="PSUM") as pro_ps:
            # qh2 = [WiWq^T @ cand_even + c0 ; WiWq^T @ cand_odd + c0]
            qp = pro_ps.tile([2 * A, NPAIR], F32, tag="pps")
            nc.tensor.matmul(qp, wiq2a, candT_eo[:, 0:NPAIR],
                             start=True, stop=False)
            nc.tensor.matmul(qp, wiq2b, candT_eo[:, NPAIR:nb],
                             start=False, stop=True)
            nc.vector.tensor_scalar_add(qh2, qp, c0_2)

            # cand embedding (ascending order): cembT = Wi^T @ candT + bi
            for m in range(nb // NCH):
                cols = slice(m * NCH, (m + 1) * NCH)
                cep = pro_ps.tile([E, NCH], F32, tag="pps")
                nc.tensor.matmul(cep, wi, candT_asc[:, cols], start=True, stop=True)
                nc.scalar.activation(cembT[:, cols], cep, AF.Identity, bias=bi2)

        # ml pooling weights into poolW odd columns
        nc.vector.tensor_copy(poolW0[:, :, 1], mlT0)
        nc.vector.tensor_copy(poolW1[:, :, 1], mlT1)

        # ---------------- score pipeline ----------------
        score_ps = ctx.enter_context(contextlib.ExitStack())
        kh_ps = score_ps.enter_context(tc.tile_pool(name="kh_ps", bufs=2, space="PSUM"))
        sc_ps = score_ps.enter_context(tc.tile_pool(name="sc_ps", bufs=2, space="PSUM"))
        tr_ps = score_ps.enter_context(tc.tile_pool(name="tr_ps", bufs=1, space="PSUM"))
        at_ps = score_ps.enter_context(tc.tile_pool(name="at_ps", bufs=1, space="PSUM"))
        for g in range(G):
            scT = sc_ps.tile([128, 512], F32, tag="scT")
            for q8 in range(8):
                b0 = g * 128 + q8 * 16
                ht = histT_p.tile([F, 16, S], BF, tag="ht")
                nc.sync.dma_start(out=ht, in_=d["histT"][:, b0:b0 + 16, :])
                for i in range(8):
                    pp = q8 * 8 + i          # pair within group
                    t = g * 64 + pp          # global pair
                    khp = kh_ps.tile([2 * A, S], F32, tag="khp")
                    nc.tensor.matmul(khp, wik2a, ht[:, 2 * i, :],
                                     start=True, stop=False)
                    nc.tensor.matmul(khp, wik2b, ht[:, 2 * i + 1, :],
                                     start=False, stop=True)
                    hid = hid_p.tile([2 * A, S], BF, tag="hid")
                    if t % 2 == 0:
                        nc.scalar.activation(hid, khp, AF.Relu,
                                             bias=qh2[:, t:t + 1])
                    else:
                        nc.vector.tensor_scalar(hid, khp, qh2[:, t:t + 1], 0.0,
                                                ALU.add, ALU.max)
                    # scores^T: [s, 2] per pair
                    nc.tensor.matmul(scT[0:S0, 2 * pp:2 * pp + 2],
                                     hid[:, 0:S0], wv2, start=True, stop=True)
                    nc.tensor.matmul(scT[0:S1, 128 + 2 * pp:130 + 2 * pp],
                                     hid[:, S0:S], wv2, start=True, stop=True)

            # drain scores^T, transpose to [b, s], masked softmax
            scT0 = grp_p.tile([S0, 128], F32, tag="scT0")
            scT1 = grp_p.tile([S1, 128], F32, tag="scT1")
            nc.vector.tensor_copy(scT0, scT[:, 0:128])
            nc.vector.tensor_copy(scT1, scT[0:S1, 128:256])
            scps = tr_ps.tile([128, S], F32, tag="scps")
            nc.tensor.transpose(scps[:, 0:S0], scT0, identf)
            nc.tensor.transpose(scps[:, S0:S], scT1, identf[0:S1, 0:S1])
            mtile = grp_p.tile([128, S], F32, tag="mtile")
            nc.sync.dma_start(out=mtile, in_=d["maskadd"][g * 128:(g + 1) * 128, :])
            scm = grp_p.tile([128, S], F32, tag="scm")
            nc.vector.tensor_add(scm, scps, mtile)
            esc = grp_p.tile([128, S], F32, tag="esc")
            nc.scalar.activation(esc, scm, AF.Exp)
            ssum = grp_p.tile([128, 1], F32, tag="ssum")
            nc.vector.tensor_reduce(ssum, esc, axis=mybir.AxisListType.X, op=ALU.add)
            rec = grp_p.tile([128, 1], F32, tag="rec")
            nc.vector.reciprocal(rec, ssum)
            attn = grp_p.tile([128, S], BF, tag="attn")
            nc.vector.tensor_scalar_mul(attn, esc, rec)
            # transpose attn back to [s, b] and write pool weights (even cols)
            atp = at_ps.tile([128, 256], BF, tag="atp")
            nc.tensor.transpose(atp[:, 0:128], attn[:, 0:S0], identb)
            nc.tensor.transpose(atp[0:S1, 128:256], attn[:, S0:S], identb)
            gs = slice(g * 128, (g + 1) * 128)
            nc.vector.tensor_copy(poolW0[:, gs, 0], atp[:, 0:128])
            nc.vector.tensor_copy(poolW1[:, gs, 0], atp[0:S1, 128:256])

        score_ps.close()

        # ---------------- pooling ----------------
        pool_ps = ctx.enter_context(tc.tile_pool(name="pool_ps", bufs=2, space="PSUM"))
        for g in range(G):
            pps = pool_ps.tile([F, 256], F32, tag="pps")
            for q in range(4):
                b0 = g * 128 + q * 32
                n0 = nat_p.tile([S0, 32, F], BF, tag="n0")
                nc.sync.dma_start(out=n0, in_=d["nat"][0:S0, b0:b0 + 32, :])
                n1 = nat_p.tile([S1, 32, F], BF, tag="n1")
                nc.sync.dma_start(out=n1, in_=d["nat"][S0:S, b0:b0 + 32, :])
                for i in range(32):
                    j = q * 32 + i
                    b = g * 128 + j
                    nc.tensor.matmul(pps[:, 2 * j:2 * j + 2], n0[:, i, :],
                                     poolW0[:, b, :], start=True, stop=False)
                    nc.tensor.matmul(pps[:, 2 * j:2 * j + 2], n1[:, i, :],
                                     poolW1[:, b, :], start=False, stop=True)
            v = pps.rearrange("p (b j) -> p b j", j=2)
            gs = slice(g * 128, (g + 1) * 128)
            nc.vector.tensor_copy(irT[:, gs], v[:, :, 0])
            nc.vector.tensor_copy(avT[:, gs], v[:, :, 1])

        # ---------------- embed + MLP head ----------------
        mlp_ps = ctx.enter_context(tc.tile_pool(name="mlp_ps", bufs=3, space="PSUM"))
        for m in range(nb // NCH):
            cols = slice(m * NCH, (m + 1) * NCH)
            ip = mlp_ps.tile([E, NCH], F32, tag="mp")
            nc.tensor.matmul(ip, wi, irT[:, cols], start=True, stop=True)
            eiT = mlp_p.tile([E, NCH], BF, tag="eiT")
            nc.scalar.activation(eiT, ip, AF.Identity, bias=bi2)
            ap_ = mlp_ps.tile([E, NCH], F32, tag="mp")
            nc.tensor.matmul(ap_, wi, avT[:, cols], start=True, stop=True)
            eaT = mlp_p.tile([E, NCH], BF, tag="eaT")
            nc.scalar.activation(eaT, ap_, AF.Identity, bias=bi2)

            h1p = mlp_ps.tile([128, NCH], F32, tag="mp")
            nc.tensor.matmul(h1p, w1a, eiT, start=True, stop=False)
            nc.tensor.matmul(h1p, w1b, cembT[:, cols], start=False, stop=False)
            nc.tensor.matmul(h1p, w1c, eaT, start=False, stop=True)
            h1 = mlp_p.tile([128, NCH], BF, tag="h1")
            nc.scalar.activation(h1, h1p, AF.Relu, bias=bn1h, scale=bn1s)

            h2p = mlp_ps.tile([64, NCH], F32, tag="mp")
            nc.tensor.matmul(h2p, w2, h1, start=True, stop=True)
            h2 = mlp_p.tile([64, NCH], BF, tag="h2")
            nc.scalar.activation(h2, h2p, AF.Relu, bias=bn2h, scale=bn2s)

            h3p = mlp_ps.tile([32, NCH], F32, tag="mp")
            nc.tensor.matmul(h3p, w3, h2, start=True, stop=True)
            h3 = mlp_p.tile([32, NCH], BF, tag="h3")
            nc.scalar.activation(h3, h3p, AF.Relu, bias=bn3h, scale=bn3s)

            lp = mlp_ps.tile([1, NCH], F32, tag="mp")
            nc.tensor.matmul(lp, wo, h3, start=True, stop=True)
            osb = mlp_p.tile([1, NCH], F32, tag="osb")
            nc.scalar.activation(osb, lp, AF.Sigmoid, bias=bo2[0:1, 0:1])
            nc.sync.dma_start(out=out_d[cols].unsqueeze(0), in_=osb)


_PROGRAMS = {}


def _build_program(nb):
    if nb in _PROGRAMS:
        return _PROGRAMS[nb]
    nc = bacc.Bacc("TRN2", target_bir_lowering=False, debug=False)
    d = _declare_inputs(nc, nb)
    out_d = nc.dram_tensor("out", [nb], F32, kind="ExternalOutput").ap()
    with tile.TileContext(nc) as tc:
        _body(tc, d, out_d, nb)
    nc.compile()
    _PROGRAMS[nb] = nc
    return nc


# ------------------------------------------------------------------ host ----

def _prep_maps(inputs, ncores=NCORES):
    nb = inputs["history_features"].shape[0] // ncores
    hist = np.asarray(inputs["history_features"], dtype=np.float32)
    cand = np.asarray(inputs["candidate_features"], dtype=np.float32)
    ln = np.asarray(inputs["history_length"]).astype(np.int64)
    b_tot = hist.shape[0]

    Wi = np.asarray(inputs["Wi"]); bi = np.asarray(inputs["bi"])
    Wq = np.asarray(inputs["Wq"]); Wk = np.asarray(inputs["Wk"])
    Wv = np.asarray(inputs["Wv"])

    hist_bf = hist.astype(BF16NP)
    histT_g = hist_bf.transpose(2, 0, 1)          # [F, B, S]
    nat_g = hist_bf.transpose(1, 0, 2)            # [S, B, F]
    candT = cand.astype(BF16NP).T                 # [F, B]

    seq = np.arange(S)
    mask = seq[None, :] < ln[:, None]
    maskadd_g = np.where(mask, np.float32(0), np.float32(-1e9)).astype(np.float32)
    mlT_g = np.where(mask, 1.0 / ln[:, None], 0.0).astype(BF16NP).T  # [S, B]

    wik = (Wi @ Wk).astype(BF16NP)
    wiq = (Wi @ Wq).astype(BF16NP)

    def stack2(w):
        wa = np.zeros((F, 128), dtype=BF16NP)
        wb = np.zeros((F, 128), dtype=BF16NP)
        wa[:, 0:A] = w
        wb[:, A:128] = w
        return wa, wb

    wik2a, wik2b = stack2(wik)
    wiq2a, wiq2b = stack2(wiq)
    c0 = (bi @ Wk + bi @ Wq).astype(np.float32)
    wv2 = np.zeros((2 * A, 2), dtype=BF16NP)
    wv2[0:A, 0] = Wv.astype(BF16NP)
    wv2[A:2 * A, 1] = Wv.astype(BF16NP)

    def bn_fold(b_, g_, be, m, v):
        s_ = (g_ / np.sqrt(v + EPS)).astype(np.float32)
        h_ = ((b_ - m) * s_ + be).astype(np.float32)
        return s_, h_

    s1, h1 = bn_fold(*(np.asarray(inputs[k]) for k in ("b1", "g1", "be1", "m1", "v1")))
    s2, h2 = bn_fold(*(np.asarray(inputs[k]) for k in ("b2", "g2", "be2", "m2", "v2")))
    s3, h3 = bn_fold(*(np.asarray(inputs[k]) for k in ("b3", "g3", "be3", "m3", "v3")))
    W1 = np.asarray(inputs["W1"])

    shared = {
        "wik2a": wik2a, "wik2b": wik2b, "wiq2a": wiq2a, "wiq2b": wiq2b,
        "wi": Wi.astype(BF16NP),
        "wv2": wv2,
        "c0_2": np.concatenate([c0, c0]).reshape(2 * A, 1),
        "bi2": bi.reshape(E, 1).astype(np.float32),
        "identf": np.eye(128, dtype=np.float32),
        "identb": np.eye(128, dtype=np.float32).astype(BF16NP),
        "w1a": W1[0:64].astype(BF16NP), "w1b": W1[64:128].astype(BF16NP),
        "w1c": W1[128:192].astype(BF16NP),
        "bn1s": s1.reshape(-1, 1), "bn1h": h1.reshape(-1, 1),
        "w2": np.asarray(inputs["W2"]).astype(BF16NP),
        "bn2s": s2.reshape(-1, 1), "bn2h": h2.reshape(-1, 1),
        "w3": np.asarray(inputs["W3"]).astype(BF16NP),
        "bn3s": s3.reshape(-1, 1), "bn3h": h3.reshape(-1, 1),
        "wo": np.asarray(inputs["Wo"]).astype(BF16NP),
        "bo2": np.asarray(inputs["bo"]).reshape(1, 1).astype(np.float32),
    }

    in_maps = []
    for c in range(ncores):
        cs = slice(c * nb, (c + 1) * nb)
        ct = np.ascontiguousarray(candT[:, cs])
        m = dict(shared)
        m["histT"] = np.ascontiguousarray(histT_g[:, cs, :])
        m["nat"] = np.ascontiguousarray(nat_g[:, cs, :])
        m["maskadd"] = np.ascontiguousarray(maskadd_g[cs])
        m["mlT"] = np.ascontiguousarray(mlT_g[:, cs])
        m["candT_eo"] = np.concatenate([ct[:, 0::2], ct[:, 1::2]], axis=1)
        m["candT_asc"] = ct
        in_maps.append(m)
    return nb, in_maps


LAST_RESULT = None


def kernel(**inputs):
    global LAST_RESULT
    import os
    nb, in_maps = _prep_maps(inputs)
    nc = _build_program(nb)
    trace = bool(int(os.environ.get("DIN_TRACE", "0")))
    res = bass_utils.run_bass_kernel_spmd(
        nc, in_maps, core_ids=list(range(NCORES)), trace=trace)
    LAST_RESULT = res
    out = np.concatenate([r["out"] for r in res.results])
    return out.astype(np.float32)


# revision 12
# speedup vs baseline: 1.3020x; 1.3020x over previous
"""DIN (deep interest network) forward pass on 8 Trainium2 NeuronCores.

Strategy: pure data-parallel over the batch (1024 rows/core). The large
history tensor is staged host-side into two bf16 layouts -- [F, B, S]
(feature-major, feeds the attention-score matmuls) and [S, B, F]
(seq-major, feeds the attention/mean pooling matmuls) -- so the device
never transposes the big tensor.  Total HBM traffic per core matches a
single fp32 load of the original tensor (~105 MB).

All matmuls contract on the partition dim with fp32 PSUM accumulation;
weights are pre-folded host-side (Wi@Wk, Wi@Wq, batchnorm scale/shift).
"""
import sys
import numpy as np

sys.path.insert(0, "/opt/trn_rl_repo")

import concourse.bass as bass
import concourse.bacc as bacc
import concourse.tile as tile
from concourse import mybir
from concourse import bass_utils
import ml_dtypes

BF16NP = ml_dtypes.bfloat16
F32 = mybir.dt.float32
BF = mybir.dt.bfloat16
AF = mybir.ActivationFunctionType
ALU = mybir.AluOpType

NCORES = 8
B, S, F, E, A = 8192, 200, 128, 64, 64
NB = B // NCORES          # batch rows per core
EPS = 1e-5


# ---------------------------------------------------------------- device ----

def _declare_inputs(nc, nb):
    d = {}

    def din(name, shape, dt):
        d[name] = nc.dram_tensor(name, shape, dt, kind="ExternalInput").ap()

    nbp = nb + 1                        # pad row: de-alias HBM channel stride
    din("histT", [F, nbp, S], BF)       # feature-major history
    din("nat", [S, nbp, F], BF)         # seq-major history
    din("maskadd", [nb, S], F32)        # 0 / -1e9 additive score mask
    din("mlT", [S, nb], BF)             # (s<len)/len pooling weights, seq-major
    din("candT_eo", [F, nb], BF)        # cand^T, even b in first half, odd in second
    din("candT_asc", [F, nb], BF)       # cand^T, ascending b
    din("wik2a", [F, 128], BF)          # [Wi@Wk | 0]
    din("wik2b", [F, 128], BF)          # [0 | Wi@Wk]
    din("wiq2a", [F, 128], BF)          # [Wi@Wq | 0]
    din("wiq2b", [F, 128], BF)          # [0 | Wi@Wq]
    din("wi", [F, E], BF)
    din("wv2", [2 * A, 2], BF)          # blockdiag([Wv],[Wv])
    din("c0_2", [2 * A, 1], F32)        # [bi@Wk + bi@Wq] stacked twice
    din("bi2", [E, 1], F32)
    din("identf", [128, 128], F32)
    din("identb", [128, 128], BF)
    din("w1a", [64, 128], BF)
    din("w1b", [64, 128], BF)
    din("w1c", [64, 128], BF)
    din("bn1s", [128, 1], F32)
    din("bn1h", [128, 1], F32)
    din("w2", [128, 64], BF)
    din("bn2s", [64, 1], F32)
    din("bn2h", [64, 1], F32)
    din("w3", [64, 32], BF)
    din("bn3s", [32, 1], F32)
    din("bn3h", [32, 1], F32)
    din("wo", [32, 1], BF)
    din("bo2", [1, 1], F32)
    return d


def _body(tc, d, out_d, nb):
    nc = tc.nc
    import contextlib
    ctx = contextlib.ExitStack()
    S0, S1 = 128, S - 128               # seq chunks on partitions
    G = nb // 128                       # 128-batch groups
    NPAIR = nb // 2
    with ctx:
        consts = ctx.enter_context(tc.tile_pool(name="consts", bufs=1))
        persist = ctx.enter_context(tc.tile_pool(name="persist", bufs=1))
        histT_p = ctx.enter_context(tc.tile_pool(name="histT", bufs=3))
        hid_p = ctx.enter_context(tc.tile_pool(name="hid", bufs=4))
        grp_p = ctx.enter_context(tc.tile_pool(name="grp", bufs=2))
        nat_p = ctx.enter_context(tc.tile_pool(name="nat", bufs=3))
        mlp_p = ctx.enter_context(tc.tile_pool(name="mlp", bufs=2))

        def load_const(name, shape, dt):
            t = consts.tile(shape, dt, tag=name)
            nc.sync.dma_start(out=t, in_=d[name])
            return t

        wik2a = load_const("wik2a", [F, 128], BF)
        wik2b = load_const("wik2b", [F, 128], BF)
        wiq2a = load_const("wiq2a", [F, 128], BF)
        wiq2b = load_const("wiq2b", [F, 128], BF)
        wi = load_const("wi", [F, E], BF)
        wv2 = load_const("wv2", [2 * A, 2], BF)
        c0_2 = load_const("c0_2", [2 * A, 1], F32)
        bi2 = load_const("bi2", [E, 1], F32)
        identf = load_const("identf", [128, 128], F32)
        identb = load_const("identb", [128, 128], BF)
        w1a = load_const("w1a", [64, 128], BF)
        w1b = load_const("w1b", [64, 128], BF)
        w1c = load_const("w1c", [64, 128], BF)
        bn1s = load_const("bn1s", [128, 1], F32)
        bn1h = load_const("bn1h", [128, 1], F32)
        w2 = load_const("w2", [128, 64], BF)
        bn2s = load_const("bn2s", [64, 1], F32)
        bn2h = load_const("bn2h", [64, 1], F32)
        w3 = load_const("w3", [64, 32], BF)
        bn3s = load_const("bn3s", [32, 1], F32)
        bn3h = load_const("bn3h", [32, 1], F32)
        wo = load_const("wo", [32, 1], BF)
        bo2 = load_const("bo2", [1, 1], F32)
        candT_eo = load_const("candT_eo", [F, nb], BF)
        candT_asc = load_const("candT_asc", [F, nb], BF)
        mlT0 = consts.tile([S0, nb], BF, tag="mlT0")
        nc.sync.dma_start(out=mlT0, in_=d["mlT"][0:S0, :])
        mlT1 = consts.tile([S1, nb], BF, tag="mlT1")
        nc.sync.dma_start(out=mlT1, in_=d["mlT"][S0:S, :])

        # persistent buffers
        qh2 = persist.tile([2 * A, NPAIR], F32, tag="qh2")
        cembT = persist.tile([E, nb], BF, tag="cembT")
        poolW0 = persist.tile([S0, nb, 2], BF, tag="poolW0")
        poolW1 = persist.tile([S1, nb, 2], BF, tag="poolW1")
        irT = persist.tile([F, nb], BF, tag="irT")
        avT = persist.tile([F, nb], BF, tag="avT")

        NCH = min(512, nb)
        with tc.tile_pool(name="pro_ps", bufs=2, space="PSUM") as pro_ps:
            # qh2 = [WiWq^T @ cand_even + c0 ; WiWq^T @ cand_odd + c0]
            qp = pro_ps.tile([2 * A, NPAIR], F32, tag="pps")
            nc.tensor.matmul(qp, wiq2a, candT_eo[:, 0:NPAIR],
                             start=True, stop=False)
            nc.tensor.matmul(qp, wiq2b, candT_eo[:, NPAIR:nb],
                             start=False, stop=True)
            nc.vector.tensor_scalar_add(qh2, qp, c0_2)

            # cand embedding (ascending order): cembT = Wi^T @ candT + bi
            for m in range(nb // NCH):
                cols = slice(m * NCH, (m + 1) * NCH)
                cep = pro_ps.tile([E, NCH], F32, tag="pps")
                nc.tensor.matmul(cep, wi, candT_asc[:, cols], start=True, stop=True)
                nc.scalar.activation(cembT[:, cols], cep, AF.Identity, bias=bi2)

        # ml pooling weights into poolW odd columns
        nc.vector.tensor_copy(poolW0[:, :, 1], mlT0)
        nc.vector.tensor_copy(poolW1[:, :, 1], mlT1)

        # ---------------- score + pooling pipeline (interleaved) ----------------
        pool_ps = ctx.enter_context(tc.tile_pool(name="pool_ps", bufs=2, space="PSUM"))
        score_ps = ctx.enter_context(contextlib.ExitStack())
        kh_ps = score_ps.enter_context(tc.tile_pool(name="kh_ps", bufs=3, space="PSUM"))
        sc_ps = score_ps.enter_context(tc.tile_pool(name="sc_ps", bufs=2, space="PSUM"))
        tr_ps = score_ps.enter_context(tc.tile_pool(name="tr_ps", bufs=1, space="PSUM"))
        at_ps = tr_ps

        def pool_group(g):
            pps = pool_ps.tile([F, 256], F32, tag="pps")
            for q in range(2):
                b0 = g * 128 + q * 64
                n0 = nat_p.tile([S0, 64, F], BF, tag="n0")
                nc.sync.dma_start(out=n0, in_=d["nat"][0:S0, b0:b0 + 64, :])
                n1 = nat_p.tile([S1, 64, F], BF, tag="n1")
                nc.sync.dma_start(out=n1, in_=d["nat"][S0:S, b0:b0 + 64, :])
                for i in range(64):
                    j = q * 64 + i
                    b = g * 128 + j
                    nc.tensor.matmul(pps[:, 2 * j:2 * j + 2], n0[:, i, :],
                                     poolW0[:, b, :], start=True, stop=False)
                    nc.tensor.matmul(pps[:, 2 * j:2 * j + 2], n1[:, i, :],
                                     poolW1[:, b, :], start=False, stop=True)
            v = pps.rearrange("p (b j) -> p b j", j=2)
            gs = slice(g * 128, (g + 1) * 128)
            nc.vector.tensor_copy(irT[:, gs], v[:, :, 0])
            nc.vector.tensor_copy(avT[:, gs], v[:, :, 1])

        for g in range(G):
            scT = sc_ps.tile([128, 512], F32, tag="scT")
            for q8 in range(4):
                b0 = g * 128 + q8 * 32
                ht = histT_p.tile([F, 32, S], BF, tag="ht")
                nc.sync.dma_start(out=ht, in_=d["histT"][:, b0:b0 + 32, :])
                for i in range(16):
                    pp = q8 * 16 + i         # pair within group
                    t = g * 64 + pp          # global pair
                    khp = kh_ps.tile([2 * A, S], F32, tag="khp")
                    nc.tensor.matmul(khp, wik2a, ht[:, 2 * i, :],
                                     start=True, stop=False)
                    nc.tensor.matmul(khp, wik2b, ht[:, 2 * i + 1, :],
                                     start=False, stop=True)
                    hid = hid_p.tile([2 * A, S], BF, tag="hid")
                    if t % 2 == 0:
                        nc.scalar.activation(hid, khp, AF.Relu,
                                             bias=qh2[:, t:t + 1])
                    else:
                        nc.vector.tensor_scalar(hid, khp, qh2[:, t:t + 1], 0.0,
                                                ALU.add, ALU.max)
                    # scores^T: [s, 2] per pair
                    nc.tensor.matmul(scT[0:S0, 2 * pp:2 * pp + 2],
                                     hid[:, 0:S0], wv2, start=True, stop=True)
                    nc.tensor.matmul(scT[0:S1, 128 + 2 * pp:130 + 2 * pp],
                                     hid[:, S0:S], wv2, start=True, stop=True)

            # drain scores^T, transpose to [b, s], masked softmax
            scT0 = grp_p.tile([S0, 128], F32, tag="scT0")
            scT1 = grp_p.tile([S1, 128], F32, tag="scT1")
            nc.vector.tensor_copy(scT0, scT[:, 0:128])
            nc.vector.tensor_copy(scT1, scT[0:S1, 128:256])
            scps = tr_ps.tile([128, S], F32, tag="trx")
            nc.tensor.transpose(scps[:, 0:S0], scT0, identf)
            nc.tensor.transpose(scps[:, S0:S], scT1, identf[0:S1, 0:S1])
            mtile = grp_p.tile([128, S], F32, tag="mtile")
            nc.sync.dma_start(out=mtile, in_=d["maskadd"][g * 128:(g + 1) * 128, :])
            scm = grp_p.tile([128, S], F32, tag="scm")
            nc.vector.tensor_add(scm, scps, mtile)
            esc = grp_p.tile([128, S], F32, tag="esc")
            nc.scalar.activation(esc, scm, AF.Exp)
            ssum = grp_p.tile([128, 1], F32, tag="ssum")
            nc.vector.tensor_reduce(ssum, esc, axis=mybir.AxisListType.X, op=ALU.add)
            rec = grp_p.tile([128, 1], F32, tag="rec")
            nc.vector.reciprocal(rec, ssum)
            attn = grp_p.tile([128, S], BF, tag="attn")
            nc.vector.tensor_scalar_mul(attn, esc, rec)
            # transpose attn back to [s, b] and write pool weights (even cols)
            atp = at_ps.tile([128, 256], BF, tag="trx")
            nc.tensor.transpose(atp[:, 0:128], attn[:, 0:S0], identb)
            nc.tensor.transpose(atp[0:S1, 128:256], attn[:, S0:S], identb)
            gs = slice(g * 128, (g + 1) * 128)
            nc.vector.tensor_copy(poolW0[:, gs, 0], atp[:, 0:128])
            nc.vector.tensor_copy(poolW1[:, gs, 0], atp[0:S1, 128:256])
            if g > 0:
                pool_group(g - 1)
        pool_group(G - 1)

        score_ps.close()

        # ---------------- embed + MLP head ----------------
        mlp_ps = ctx.enter_context(tc.tile_pool(name="mlp_ps", bufs=3, space="PSUM"))
        for m in range(nb // NCH):
            cols = slice(m * NCH, (m + 1) * NCH)
            ip = mlp_ps.tile([E, NCH], F32, tag="mp")
            nc.tensor.matmul(ip, wi, irT[:, cols], start=True, stop=True)
            eiT = mlp_p.tile([E, NCH], BF, tag="eiT")
            nc.scalar.activation(eiT, ip, AF.Identity, bias=bi2)
            ap_ = mlp_ps.tile([E, NCH], F32, tag="mp")
            nc.tensor.matmul(ap_, wi, avT[:, cols], start=True, stop=True)
            eaT = mlp_p.tile([E, NCH], BF, tag="eaT")
            nc.scalar.activation(eaT, ap_, AF.Identity, bias=bi2)

            h1p = mlp_ps.tile([128, NCH], F32, tag="mp")
            nc.tensor.matmul(h1p, w1a, eiT, start=True, stop=False)
            nc.tensor.matmul(h1p, w1b, cembT[:, cols], start=False, stop=False)
            nc.tensor.matmul(h1p, w1c, eaT, start=False, stop=True)
            h1 = mlp_p.tile([128, NCH], BF, tag="h1")
            nc.scalar.activation(h1, h1p, AF.Relu, bias=bn1h, scale=bn1s)

            h2p = mlp_ps.tile([64, NCH], F32, tag="mp")
            nc.tensor.matmul(h2p, w2, h1, start=True, stop=True)
            h2 = mlp_p.tile([64, NCH], BF, tag="h2")
            nc.scalar.activation(h2, h2p, AF.Relu, bias=bn2h, scale=bn2s)

            h3p = mlp_ps.tile([32, NCH], F32, tag="mp")
            nc.tensor.matmul(h3p, w3, h2, start=True, stop=True)
            h3 = mlp_p.tile([32, NCH], BF, tag="h3")
            nc.scalar.activation(h3, h3p, AF.Relu, bias=bn3h, scale=bn3s)

            lp = mlp_ps.tile([1, NCH], F32, tag="mp")
            nc.tensor.matmul(lp, wo, h3, start=True, stop=True)
            osb = mlp_p.tile([1, NCH], F32, tag="osb")
            nc.scalar.activation(osb, lp, AF.Sigmoid, bias=bo2[0:1, 0:1])
            nc.sync.dma_start(out=out_d[cols].unsqueeze(0), in_=osb)


_PROGRAMS = {}


def _build_program(nb):
    if nb in _PROGRAMS:
        return _PROGRAMS[nb]
    nc = bacc.Bacc("TRN2", target_bir_lowering=False, debug=False)
    d = _declare_inputs(nc, nb)
    out_d = nc.dram_tensor("out", [nb], F32, kind="ExternalOutput").ap()
    with tile.TileContext(nc) as tc:
        _body(tc, d, out_d, nb)
    nc.compile()
    _PROGRAMS[nb] = nc
    return nc


# ------------------------------------------------------------------ host ----

def _prep_maps(inputs, ncores=NCORES):
    nb = inputs["history_features"].shape[0] // ncores
    hist = np.asarray(inputs["history_features"], dtype=np.float32)
    cand = np.asarray(inputs["candidate_features"], dtype=np.float32)
    ln = np.asarray(inputs["history_length"]).astype(np.int64)
    b_tot = hist.shape[0]

    Wi = np.asarray(inputs["Wi"]); bi = np.asarray(inputs["bi"])
    Wq = np.asarray(inputs["Wq"]); Wk = np.asarray(inputs["Wk"])
    Wv = np.asarray(inputs["Wv"])

    hist_bf = hist.astype(BF16NP)
    histT_g = hist_bf.transpose(2, 0, 1)          # [F, B, S]
    nat_g = hist_bf.transpose(1, 0, 2)            # [S, B, F]
    candT = cand.astype(BF16NP).T                 # [F, B]

    seq = np.arange(S)
    mask = seq[None, :] < ln[:, None]
    maskadd_g = np.where(mask, np.float32(0), np.float32(-1e9)).astype(np.float32)
    mlT_g = np.where(mask, 1.0 / ln[:, None], 0.0).astype(BF16NP).T  # [S, B]

    wik = (Wi @ Wk).astype(BF16NP)
    wiq = (Wi @ Wq).astype(BF16NP)

    def stack2(w):
        wa = np.zeros((F, 128), dtype=BF16NP)
        wb = np.zeros((F, 128), dtype=BF16NP)
        wa[:, 0:A] = w
        wb[:, A:128] = w
        return wa, wb

    wik2a, wik2b = stack2(wik)
    wiq2a, wiq2b = stack2(wiq)
    c0 = (bi @ Wk + bi @ Wq).astype(np.float32)
    wv2 = np.zeros((2 * A, 2), dtype=BF16NP)
    wv2[0:A, 0] = Wv.astype(BF16NP)
    wv2[A:2 * A, 1] = Wv.astype(BF16NP)

    def bn_fold(b_, g_, be, m, v):
        s_ = (g_ / np.sqrt(v + EPS)).astype(np.float32)
        h_ = ((b_ - m) * s_ + be).astype(np.float32)
        return s_, h_

    s1, h1 = bn_fold(*(np.asarray(inputs[k]) for k in ("b1", "g1", "be1", "m1", "v1")))
    s2, h2 = bn_fold(*(np.asarray(inputs[k]) for k in ("b2", "g2", "be2", "m2", "v2")))
    s3, h3 = bn_fold(*(np.asarray(inputs[k]) for k in ("b3", "g3", "be3", "m3", "v3")))
    W1 = np.asarray(inputs["W1"])

    shared = {
        "wik2a": wik2a, "wik2b": wik2b, "wiq2a": wiq2a, "wiq2b": wiq2b,
        "wi": Wi.astype(BF16NP),
        "wv2": wv2,
        "c0_2": np.concatenate([c0, c0]).reshape(2 * A, 1),
        "bi2": bi.reshape(E, 1).astype(np.float32),
        "identf": np.eye(128, dtype=np.float32),
        "identb": np.eye(128, dtype=np.float32).astype(BF16NP),
        "w1a": W1[0:64].astype(BF16NP), "w1b": W1[64:128].astype(BF16NP),
        "w1c": W1[128:192].astype(BF16NP),
        "bn1s": s1.reshape(-1, 1), "bn1h": h1.reshape(-1, 1),
        "w2": np.asarray(inputs["W2"]).astype(BF16NP),
        "bn2s": s2.reshape(-1, 1), "bn2h": h2.reshape(-1, 1),
        "w3": np.asarray(inputs["W3"]).astype(BF16NP),
        "bn3s": s3.reshape(-1, 1), "bn3h": h3.reshape(-1, 1),
        "wo": np.asarray(inputs["Wo"]).astype(BF16NP),
        "bo2": np.asarray(inputs["bo"]).reshape(1, 1).astype(np.float32),
    }

    in_maps = []
    for c in range(ncores):
        cs = slice(c * nb, (c + 1) * nb)
        ct = np.ascontiguousarray(candT[:, cs])
        m = dict(shared)
        ht = np.empty((F, nb + 1, S), dtype=BF16NP)
        ht[:, 0:nb, :] = histT_g[:, cs, :]
        m["histT"] = ht
        nt = np.empty((S, nb + 1, F), dtype=BF16NP)
        nt[:, 0:nb, :] = nat_g[:, cs, :]
        m["nat"] = nt
        m["maskadd"] = np.ascontiguousarray(maskadd_g[cs])
        m["mlT"] = np.ascontiguousarray(mlT_g[:, cs])
        m["candT_eo"] = np.concatenate([ct[:, 0::2], ct[:, 1::2]], axis=1)
        m["candT_asc"] = ct
        in_maps.append(m)
    return nb, in_maps


LAST_RESULT = None


def kernel(**inputs):
    global LAST_RESULT
    import os
    nb, in_maps = _prep_maps(inputs)
    nc = _build_program(nb)
    trace = bool(int(os.environ.get("DIN_TRACE", "0")))
    res = bass_utils.run_bass_kernel_spmd(
        nc, in_maps, core_ids=list(range(NCORES)), trace=trace)
    LAST_RESULT = res
    out = np.concatenate([r["out"] for r in res.results])
    return out.astype(np.float32)


# revision 13
# speedup vs baseline: 1.3061x; 1.0031x over previous
"""DIN (deep interest network) forward pass on 8 Trainium2 NeuronCores.

Strategy: pure data-parallel over the batch (1024 rows/core). The large
history tensor is staged host-side into two bf16 layouts -- [F, B, S]
(feature-major, feeds the attention-score matmuls) and [S, B, F]
(seq-major, feeds the attention/mean pooling matmuls) -- so the device
never transposes the big tensor.  Total HBM traffic per core matches a
single fp32 load of the original tensor (~105 MB).

All matmuls contract on the partition dim with fp32 PSUM accumulation;
weights are pre-folded host-side (Wi@Wk, Wi@Wq, batchnorm scale/shift).
"""
import sys
import numpy as np

sys.path.insert(0, "/opt/trn_rl_repo")

import concourse.bass as bass
import concourse.bacc as bacc
import concourse.tile as tile
from concourse import mybir
from concourse import bass_utils
import ml_dtypes

BF16NP = ml_dtypes.bfloat16
F32 = mybir.dt.float32
BF = mybir.dt.bfloat16
AF = mybir.ActivationFunctionType
ALU = mybir.AluOpType

NCORES = 8
B, S, F, E, A = 8192, 200, 128, 64, 64
NB = B // NCORES          # batch rows per core
EPS = 1e-5


# ---------------------------------------------------------------- device ----

def _declare_inputs(nc, nb):
    d = {}

    def din(name, shape, dt):
        d[name] = nc.dram_tensor(name, shape, dt, kind="ExternalInput").ap()

    nbp = nb + 1                        # pad row: de-alias HBM channel stride
    din("histT", [F, nbp, S], BF)       # feature-major history
    din("nat", [S, nbp, F], BF)         # seq-major history
    din("maskadd", [nb, S], F32)        # 0 / -1e9 additive score mask
    din("mlT", [S, nb], BF)             # (s<len)/len pooling weights, seq-major
    din("candT_eo", [F, nb], BF)        # cand^T, even b in first half, odd in second
    din("candT_asc", [F, nb], BF)       # cand^T, ascending b
    din("wik2a", [F, 128], BF)          # [Wi@Wk | 0]
    din("wik2b", [F, 128], BF)          # [0 | Wi@Wk]
    din("wiq2a", [F, 128], BF)          # [Wi@Wq | 0]
    din("wiq2b", [F, 128], BF)          # [0 | Wi@Wq]
    din("wi", [F, E], BF)
    din("wv2", [2 * A, 2], BF)          # blockdiag([Wv],[Wv])
    din("c0_2", [2 * A, 1], F32)        # [bi@Wk + bi@Wq] stacked twice
    din("bi2", [E, 1], F32)
    din("identf", [128, 128], F32)
    din("identb", [128, 128], BF)
    din("w1a", [64, 128], BF)
    din("w1b", [64, 128], BF)
    din("w1c", [64, 128], BF)
    din("bn1s", [128, 1], F32)
    din("bn1h", [128, 1], F32)
    din("w2", [128, 64], BF)
    din("bn2s", [64, 1], F32)
    din("bn2h", [64, 1], F32)
    din("w3", [64, 32], BF)
    din("bn3s", [32, 1], F32)
    din("bn3h", [32, 1], F32)
    din("wo", [32, 1], BF)
    din("bo2", [1, 1], F32)
    return d


def _body(tc, d, out_d, nb):
    nc = tc.nc
    import contextlib
    ctx = contextlib.ExitStack()
    S0, S1 = 128, S - 128               # seq chunks on partitions
    G = nb // 128                       # 128-batch groups
    NPAIR = nb // 2
    with ctx:
        consts = ctx.enter_context(tc.tile_pool(name="consts", bufs=1))
        persist = ctx.enter_context(tc.tile_pool(name="persist", bufs=1))
        histT_p = ctx.enter_context(tc.tile_pool(name="histT", bufs=3))
        hid_p = ctx.enter_context(tc.tile_pool(name="hid", bufs=4))
        grp_p = ctx.enter_context(tc.tile_pool(name="grp", bufs=2))
        nat_p = ctx.enter_context(tc.tile_pool(name="nat", bufs=3))
        mlp_p = ctx.enter_context(tc.tile_pool(name="mlp", bufs=2))

        def load_const(name, shape, dt):
            t = consts.tile(shape, dt, tag=name)
            nc.sync.dma_start(out=t, in_=d[name])
            return t

        wik2a = load_const("wik2a", [F, 128], BF)
        wik2b = load_const("wik2b", [F, 128], BF)
        wiq2a = load_const("wiq2a", [F, 128], BF)
        wiq2b = load_const("wiq2b", [F, 128], BF)
        wi = load_const("wi", [F, E], BF)
        wv2 = load_const("wv2", [2 * A, 2], BF)
        c0_2 = load_const("c0_2", [2 * A, 1], F32)
        bi2 = load_const("bi2", [E, 1], F32)
        identf = load_const("identf", [128, 128], F32)
        identb = load_const("identb", [128, 128], BF)
        w1a = load_const("w1a", [64, 128], BF)
        w1b = load_const("w1b", [64, 128], BF)
        w1c = load_const("w1c", [64, 128], BF)
        bn1s = load_const("bn1s", [128, 1], F32)
        bn1h = load_const("bn1h", [128, 1], F32)
        w2 = load_const("w2", [128, 64], BF)
        bn2s = load_const("bn2s", [64, 1], F32)
        bn2h = load_const("bn2h", [64, 1], F32)
        w3 = load_const("w3", [64, 32], BF)
        bn3s = load_const("bn3s", [32, 1], F32)
        bn3h = load_const("bn3h", [32, 1], F32)
        wo = load_const("wo", [32, 1], BF)
        bo2 = load_const("bo2", [1, 1], F32)
        candT_eo = load_const("candT_eo", [F, nb], BF)
        candT_asc = load_const("candT_asc", [F, nb], BF)
        mlT0 = consts.tile([S0, nb], BF, tag="mlT0")
        nc.scalar.dma_start(out=mlT0, in_=d["mlT"][0:S0, :])
        mlT1 = consts.tile([S1, nb], BF, tag="mlT1")
        nc.scalar.dma_start(out=mlT1, in_=d["mlT"][S0:S, :])

        # persistent buffers
        qh2 = persist.tile([2 * A, NPAIR], F32, tag="qh2")
        cembT = persist.tile([E, nb], BF, tag="cembT")
        poolW0 = persist.tile([S0, nb, 2], BF, tag="poolW0")
        poolW1 = persist.tile([S1, nb, 2], BF, tag="poolW1")
        irT = persist.tile([F, nb], BF, tag="irT")
        avT = persist.tile([F, nb], BF, tag="avT")

        NCH = min(512, nb)
        with tc.tile_pool(name="pro_ps", bufs=2, space="PSUM") as pro_ps:
            # qh2 = [WiWq^T @ cand_even + c0 ; WiWq^T @ cand_odd + c0]
            qp = pro_ps.tile([2 * A, NPAIR], F32, tag="pps")
            nc.tensor.matmul(qp, wiq2a, candT_eo[:, 0:NPAIR],
                             start=True, stop=False)
            nc.tensor.matmul(qp, wiq2b, candT_eo[:, NPAIR:nb],
                             start=False, stop=True)
            nc.vector.tensor_scalar_add(qh2, qp, c0_2)

            # cand embedding (ascending order): cembT = Wi^T @ candT + bi
            for m in range(nb // NCH):
                cols = slice(m * NCH, (m + 1) * NCH)
                cep = pro_ps.tile([E, NCH], F32, tag="pps")
                nc.tensor.matmul(cep, wi, candT_asc[:, cols], start=True, stop=True)
                nc.scalar.activation(cembT[:, cols], cep, AF.Identity, bias=bi2)

        # ml pooling weights into poolW odd columns
        nc.vector.tensor_copy(poolW0[:, :, 1], mlT0)
        nc.vector.tensor_copy(poolW1[:, :, 1], mlT1)

        # ---------------- score + pooling pipeline (interleaved) ----------------
        pool_ps = ctx.enter_context(tc.tile_pool(name="pool_ps", bufs=2, space="PSUM"))
        score_ps = ctx.enter_context(contextlib.ExitStack())
        kh_ps = score_ps.enter_context(tc.tile_pool(name="kh_ps", bufs=3, space="PSUM"))
        sc_ps = score_ps.enter_context(tc.tile_pool(name="sc_ps", bufs=2, space="PSUM"))
        tr_ps = score_ps.enter_context(tc.tile_pool(name="tr_ps", bufs=1, space="PSUM"))
        at_ps = tr_ps

        def pool_group(g):
            pps = pool_ps.tile([F, 256], F32, tag="pps")
            for q in range(2):
                b0 = g * 128 + q * 64
                n0 = nat_p.tile([S0, 64, F], BF, tag="n0")
                nc.scalar.dma_start(out=n0, in_=d["nat"][0:S0, b0:b0 + 64, :])
                n1 = nat_p.tile([S1, 64, F], BF, tag="n1")
                nc.scalar.dma_start(out=n1, in_=d["nat"][S0:S, b0:b0 + 64, :])
                for i in range(64):
                    j = q * 64 + i
                    b = g * 128 + j
                    nc.tensor.matmul(pps[:, 2 * j:2 * j + 2], n0[:, i, :],
                                     poolW0[:, b, :], start=True, stop=False)
                    nc.tensor.matmul(pps[:, 2 * j:2 * j + 2], n1[:, i, :],
                                     poolW1[:, b, :], start=False, stop=True)
            v = pps.rearrange("p (b j) -> p b j", j=2)
            gs = slice(g * 128, (g + 1) * 128)
            nc.vector.tensor_copy(irT[:, gs], v[:, :, 0])
            nc.vector.tensor_copy(avT[:, gs], v[:, :, 1])

        for g in range(G):
            scT = sc_ps.tile([128, 512], F32, tag="scT")
            for q8 in range(4):
                b0 = g * 128 + q8 * 32
                ht = histT_p.tile([F, 32, S], BF, tag="ht")
                nc.sync.dma_start(out=ht, in_=d["histT"][:, b0:b0 + 32, :])
                for i in range(16):
                    pp = q8 * 16 + i         # pair within group
                    t = g * 64 + pp          # global pair
                    khp = kh_ps.tile([2 * A, S], F32, tag="khp")
                    nc.tensor.matmul(khp, wik2a, ht[:, 2 * i, :],
                                     start=True, stop=False)
                    nc.tensor.matmul(khp, wik2b, ht[:, 2 * i + 1, :],
                                     start=False, stop=True)
                    hid = hid_p.tile([2 * A, S], BF, tag="hid")
                    if t % 2 == 0:
                        nc.scalar.activation(hid, khp, AF.Relu,
                                             bias=qh2[:, t:t + 1])
                    else:
                        nc.vector.tensor_scalar(hid, khp, qh2[:, t:t + 1], 0.0,
                                                ALU.add, ALU.max)
                    # scores^T: [s, 2] per pair
                    nc.tensor.matmul(scT[0:S0, 2 * pp:2 * pp + 2],
                                     hid[:, 0:S0], wv2, start=True, stop=True)
                    nc.tensor.matmul(scT[0:S1, 128 + 2 * pp:130 + 2 * pp],
                                     hid[:, S0:S], wv2, start=True, stop=True)

            # drain scores^T, transpose to [b, s], masked softmax
            scT0 = grp_p.tile([S0, 128], F32, tag="scT0")
            scT1 = grp_p.tile([S1, 128], F32, tag="scT1")
            nc.vector.tensor_copy(scT0, scT[:, 0:128])
            nc.vector.tensor_copy(scT1, scT[0:S1, 128:256])
            scps = tr_ps.tile([128, S], F32, tag="trx")
            nc.tensor.transpose(scps[:, 0:S0], scT0, identf)
            nc.tensor.transpose(scps[:, S0:S], scT1, identf[0:S1, 0:S1])
            mtile = grp_p.tile([128, S], F32, tag="mtile")
            nc.sync.dma_start(out=mtile, in_=d["maskadd"][g * 128:(g + 1) * 128, :])
            scm = grp_p.tile([128, S], F32, tag="scm")
            nc.vector.tensor_add(scm, scps, mtile)
            esc = grp_p.tile([128, S], F32, tag="esc")
            nc.scalar.activation(esc, scm, AF.Exp)
            ssum = grp_p.tile([128, 1], F32, tag="ssum")
            nc.vector.tensor_reduce(ssum, esc, axis=mybir.AxisListType.X, op=ALU.add)
            rec = grp_p.tile([128, 1], F32, tag="rec")
            nc.vector.reciprocal(rec, ssum)
            attn = grp_p.tile([128, S], BF, tag="attn")
            nc.vector.tensor_scalar_mul(attn, esc, rec)
            # transpose attn back to [s, b] and write pool weights (even cols)
            atp = at_ps.tile([128, 256], BF, tag="trx")
            nc.tensor.transpose(atp[:, 0:128], attn[:, 0:S0], identb)
            nc.tensor.transpose(atp[0:S1, 128:256], attn[:, S0:S], identb)
            gs = slice(g * 128, (g + 1) * 128)
            nc.vector.tensor_copy(poolW0[:, gs, 0], atp[:, 0:128])
            nc.vector.tensor_copy(poolW1[:, gs, 0], atp[0:S1, 128:256])
            if g > 0:
                pool_group(g - 1)
        pool_group(G - 1)

        score_ps.close()

        # ---------------- embed + MLP head ----------------
        mlp_ps = ctx.enter_context(tc.tile_pool(name="mlp_ps", bufs=3, space="PSUM"))
        for m in range(nb // NCH):
            cols = slice(m * NCH, (m + 1) * NCH)
            ip = mlp_ps.tile([E, NCH], F32, tag="mp")
            nc.tensor.matmul(ip, wi, irT[:, cols], start=True, stop=True)
            eiT = mlp_p.tile([E, NCH], BF, tag="eiT")
            nc.scalar.activation(eiT, ip, AF.Identity, bias=bi2)
            ap_ = mlp_ps.tile([E, NCH], F32, tag="mp")
            nc.tensor.matmul(ap_, wi, avT[:, cols], start=True, stop=True)
            eaT = mlp_p.tile([E, NCH], BF, tag="eaT")
            nc.scalar.activation(eaT, ap_, AF.Identity, bias=bi2)

            h1p = mlp_ps.tile([128, NCH], F32, tag="mp")
            nc.tensor.matmul(h1p, w1a, eiT, start=True, stop=False)
            nc.tensor.matmul(h1p, w1b, cembT[:, cols], start=False, stop=False)
            nc.tensor.matmul(h1p, w1c, eaT, start=False, stop=True)
            h1 = mlp_p.tile([128, NCH], BF, tag="h1")
            nc.scalar.activation(h1, h1p, AF.Relu, bias=bn1h, scale=bn1s)

            h2p = mlp_ps.tile([64, NCH], F32, tag="mp")
            nc.tensor.matmul(h2p, w2, h1, start=True, stop=True)
            h2 = mlp_p.tile([64, NCH], BF, tag="h2")
            nc.scalar.activation(h2, h2p, AF.Relu, bias=bn2h, scale=bn2s)

            h3p = mlp_ps.tile([32, NCH], F32, tag="mp")
            nc.tensor.matmul(h3p, w3, h2, start=True, stop=True)
            h3 = mlp_p.tile([32, NCH], BF, tag="h3")
            nc.scalar.activation(h3, h3p, AF.Relu, bias=bn3h, scale=bn3s)

            lp = mlp_ps.tile([1, NCH], F32, tag="mp")
            nc.tensor.matmul(lp, wo, h3, start=True, stop=True)
            osb = mlp_p.tile([1, NCH], F32, tag="osb")
            nc.scalar.activation(osb, lp, AF.Sigmoid, bias=bo2[0:1, 0:1])
            nc.sync.dma_start(out=out_d[cols].unsqueeze(0), in_=osb)


_PROGRAMS = {}


def _build_program(nb):
    if nb in _PROGRAMS:
        return _PROGRAMS[nb]
    nc = bacc.Bacc("TRN2", target_bir_lowering=False, debug=False)
    d = _declare_inputs(nc, nb)
    out_d = nc.dram_tensor("out", [nb], F32, kind="ExternalOutput").ap()
    with tile.TileContext(nc) as tc:
        _body(tc, d, out_d, nb)
    nc.compile()
    _PROGRAMS[nb] = nc
    return nc


# ------------------------------------------------------------------ host ----

def _prep_maps(inputs, ncores=NCORES):
    nb = inputs["history_features"].shape[0] // ncores
    hist = np.asarray(inputs["history_features"], dtype=np.float32)
    cand = np.asarray(inputs["candidate_features"], dtype=np.float32)
    ln = np.asarray(inputs["history_length"]).astype(np.int64)
    b_tot = hist.shape[0]

    Wi = np.asarray(inputs["Wi"]); bi = np.asarray(inputs["bi"])
    Wq = np.asarray(inputs["Wq"]); Wk = np.asarray(inputs["Wk"])
    Wv = np.asarray(inputs["Wv"])

    hist_bf = hist.astype(BF16NP)
    histT_g = hist_bf.transpose(2, 0, 1)          # [F, B, S]
    nat_g = hist_bf.transpose(1, 0, 2)            # [S, B, F]
    candT = cand.astype(BF16NP).T                 # [F, B]

    seq = np.arange(S)
    mask = seq[None, :] < ln[:, None]
    maskadd_g = np.where(mask, np.float32(0), np.float32(-1e9)).astype(np.float32)
    mlT_g = np.where(mask, 1.0 / ln[:, None], 0.0).astype(BF16NP).T  # [S, B]

    wik = (Wi @ Wk).astype(BF16NP)
    wiq = (Wi @ Wq).astype(BF16NP)

    def stack2(w):
        wa = np.zeros((F, 128), dtype=BF16NP)
        wb = np.zeros((F, 128), dtype=BF16NP)
        wa[:, 0:A] = w
        wb[:, A:128] = w
        return wa, wb

    wik2a, wik2b = stack2(wik)
    wiq2a, wiq2b = stack2(wiq)
    c0 = (bi @ Wk + bi @ Wq).astype(np.float32)
    wv2 = np.zeros((2 * A, 2), dtype=BF16NP)
    wv2[0:A, 0] = Wv.astype(BF16NP)
    wv2[A:2 * A, 1] = Wv.astype(BF16NP)

    def bn_fold(b_, g_, be, m, v):
        s_ = (g_ / np.sqrt(v + EPS)).astype(np.float32)
        h_ = ((b_ - m) * s_ + be).astype(np.float32)
        return s_, h_

    s1, h1 = bn_fold(*(np.asarray(inputs[k]) for k in ("b1", "g1", "be1", "m1", "v1")))
    s2, h2 = bn_fold(*(np.asarray(inputs[k]) for k in ("b2", "g2", "be2", "m2", "v2")))
    s3, h3 = bn_fold(*(np.asarray(inputs[k]) for k in ("b3", "g3", "be3", "m3", "v3")))
    W1 = np.asarray(inputs["W1"])

    shared = {
        "wik2a": wik2a, "wik2b": wik2b, "wiq2a": wiq2a, "wiq2b": wiq2b,
        "wi": Wi.astype(BF16NP),
        "wv2": wv2,
        "c0_2": np.concatenate([c0, c0]).reshape(2 * A, 1),
        "bi2": bi.reshape(E, 1).astype(np.float32),
        "identf": np.eye(128, dtype=np.float32),
        "identb": np.eye(128, dtype=np.float32).astype(BF16NP),
        "w1a": W1[0:64].astype(BF16NP), "w1b": W1[64:128].astype(BF16NP),
        "w1c": W1[128:192].astype(BF16NP),
        "bn1s": s1.reshape(-1, 1), "bn1h": h1.reshape(-1, 1),
        "w2": np.asarray(inputs["W2"]).astype(BF16NP),
        "bn2s": s2.reshape(-1, 1), "bn2h": h2.reshape(-1, 1),
        "w3": np.asarray(inputs["W3"]).astype(BF16NP),
        "bn3s": s3.reshape(-1, 1), "bn3h": h3.reshape(-1, 1),
        "wo": np.asarray(inputs["Wo"]).astype(BF16NP),
        "bo2": np.asarray(inputs["bo"]).reshape(1, 1).astype(np.float32),
    }

    in_maps = []
    for c in range(ncores):
        cs = slice(c * nb, (c + 1) * nb)
        ct = np.ascontiguousarray(candT[:, cs])
        m = dict(shared)
        ht = np.empty((F, nb + 1, S), dtype=BF16NP)
        ht[:, 0:nb, :] = histT_g[:, cs, :]
        m["histT"] = ht
        nt = np.empty((S, nb + 1, F), dtype=BF16NP)
        nt[:, 0:nb, :] = nat_g[:, cs, :]
        m["nat"] = nt
        m["maskadd"] = np.ascontiguousarray(maskadd_g[cs])
        m["mlT"] = np.ascontiguousarray(mlT_g[:, cs])
        m["candT_eo"] = np.concatenate([ct[:, 0::2], ct[:, 1::2]], axis=1)
        m["candT_asc"] = ct
        in_maps.append(m)
    return nb, in_maps


LAST_RESULT = None


def kernel(**inputs):
    global LAST_RESULT
    import os
    nb, in_maps = _prep_maps(inputs)
    nc = _build_program(nb)
    trace = bool(int(os.environ.get("DIN_TRACE", "0")))
    res = bass_utils.run_bass_kernel_spmd(
        nc, in_maps, core_ids=list(range(NCORES)), trace=trace)
    LAST_RESULT = res
    out = np.concatenate([r["out"] for r in res.results])
    return out.astype(np.float32)


# revision 14
# speedup vs baseline: 1.8051x; 1.3821x over previous
"""DIN (deep interest network) forward pass on 8 Trainium2 NeuronCores.

Strategy: pure data-parallel over the batch (1024 rows/core). The large
history tensor is staged host-side into two bf16 layouts -- [F, B, S]
(feature-major, feeds the attention-score matmuls) and [S, B, F]
(seq-major, feeds the attention/mean pooling matmuls) -- so the device
never transposes the big tensor.  Total HBM traffic per core matches a
single fp32 load of the original tensor (~105 MB).

All matmuls contract on the partition dim with fp32 PSUM accumulation;
weights are pre-folded host-side (Wi@Wk, Wi@Wq, batchnorm scale/shift).
"""
import sys
import numpy as np

sys.path.insert(0, "/opt/trn_rl_repo")

import concourse.bass as bass
import concourse.bacc as bacc
import concourse.tile as tile
from concourse import mybir
from concourse import bass_utils
import ml_dtypes

BF16NP = ml_dtypes.bfloat16
F32 = mybir.dt.float32
BF = mybir.dt.bfloat16
F8 = mybir.dt.float8e4
F8NP = mybir.dt.np(mybir.dt.float8e4)
AF = mybir.ActivationFunctionType
ALU = mybir.AluOpType

NCORES = 8
B, S, F, E, A = 8192, 200, 128, 64, 64
NB = B // NCORES          # batch rows per core
EPS = 1e-5


# ---------------------------------------------------------------- device ----

def _declare_inputs(nc, nb):
    d = {}

    def din(name, shape, dt):
        d[name] = nc.dram_tensor(name, shape, dt, kind="ExternalInput").ap()

    nbp = nb + 1                        # pad row: de-alias HBM channel stride
    din("histT", [F, nbp, S], F8)       # feature-major history
    din("nat", [S, nbp, F], F8)         # seq-major history
    din("maskadd", [nb, S], F32)        # 0 / -1e9 additive score mask
    din("mlT", [S, nb], F8)             # (s<len)/len pooling weights, seq-major
    din("candT_eo", [F, nb], BF)        # cand^T, even b in first half, odd in second
    din("candT_asc", [F, nb], BF)       # cand^T, ascending b
    din("wik2a", [F, 128], F8)          # [Wi@Wk | 0]
    din("wik2b", [F, 128], F8)          # [0 | Wi@Wk]
    din("wiq2a", [F, 128], BF)          # [Wi@Wq | 0]
    din("wiq2b", [F, 128], BF)          # [0 | Wi@Wq]
    din("wi", [F, E], BF)
    din("wv2", [2 * A, 2], BF)          # blockdiag([Wv],[Wv])
    din("c0_2", [2 * A, 1], F32)        # [bi@Wk + bi@Wq] stacked twice
    din("bi2", [E, 1], F32)
    din("identf", [128, 128], F32)
    din("identb", [128, 128], BF)
    din("w1a", [64, 128], BF)
    din("w1b", [64, 128], BF)
    din("w1c", [64, 128], BF)
    din("bn1s", [128, 1], F32)
    din("bn1h", [128, 1], F32)
    din("w2", [128, 64], BF)
    din("bn2s", [64, 1], F32)
    din("bn2h", [64, 1], F32)
    din("w3", [64, 32], BF)
    din("bn3s", [32, 1], F32)
    din("bn3h", [32, 1], F32)
    din("wo", [32, 1], BF)
    din("bo2", [1, 1], F32)
    return d


def _body(tc, d, out_d, nb):
    nc = tc.nc
    import contextlib
    ctx = contextlib.ExitStack()
    S0, S1 = 128, S - 128               # seq chunks on partitions
    G = nb // 128                       # 128-batch groups
    NPAIR = nb // 2
    with ctx:
        consts = ctx.enter_context(tc.tile_pool(name="consts", bufs=1))
        persist = ctx.enter_context(tc.tile_pool(name="persist", bufs=1))
        histT_p = ctx.enter_context(tc.tile_pool(name="histT", bufs=3))
        hid_p = ctx.enter_context(tc.tile_pool(name="hid", bufs=4))
        grp_p = ctx.enter_context(tc.tile_pool(name="grp", bufs=2))
        nat_p = ctx.enter_context(tc.tile_pool(name="nat", bufs=3))
        mlp_p = ctx.enter_context(tc.tile_pool(name="mlp", bufs=2))

        def load_const(name, shape, dt):
            t = consts.tile(shape, dt, tag=name)
            nc.sync.dma_start(out=t, in_=d[name])
            return t

        wik2a = load_const("wik2a", [F, 128], F8)
        wik2b = load_const("wik2b", [F, 128], F8)
        wiq2a = load_const("wiq2a", [F, 128], BF)
        wiq2b = load_const("wiq2b", [F, 128], BF)
        wi = load_const("wi", [F, E], BF)
        wv2 = load_const("wv2", [2 * A, 2], BF)
        c0_2 = load_const("c0_2", [2 * A, 1], F32)
        bi2 = load_const("bi2", [E, 1], F32)
        identf = load_const("identf", [128, 128], F32)
        identb = load_const("identb", [128, 128], BF)
        w1a = load_const("w1a", [64, 128], BF)
        w1b = load_const("w1b", [64, 128], BF)
        w1c = load_const("w1c", [64, 128], BF)
        bn1s = load_const("bn1s", [128, 1], F32)
        bn1h = load_const("bn1h", [128, 1], F32)
        w2 = load_const("w2", [128, 64], BF)
        bn2s = load_const("bn2s", [64, 1], F32)
        bn2h = load_const("bn2h", [64, 1], F32)
        w3 = load_const("w3", [64, 32], BF)
        bn3s = load_const("bn3s", [32, 1], F32)
        bn3h = load_const("bn3h", [32, 1], F32)
        wo = load_const("wo", [32, 1], BF)
        bo2 = load_const("bo2", [1, 1], F32)
        candT_eo = load_const("candT_eo", [F, nb], BF)
        candT_asc = load_const("candT_asc", [F, nb], BF)
        mlT0 = consts.tile([S0, nb], F8, tag="mlT0")
        nc.scalar.dma_start(out=mlT0, in_=d["mlT"][0:S0, :])
        mlT1 = consts.tile([S1, nb], F8, tag="mlT1")
        nc.scalar.dma_start(out=mlT1, in_=d["mlT"][S0:S, :])

        # persistent buffers
        qh2 = persist.tile([2 * A, NPAIR], F32, tag="qh2")
        cembT = persist.tile([E, nb], BF, tag="cembT")
        poolW0 = persist.tile([S0, nb, 2], F8, tag="poolW0")
        poolW1 = persist.tile([S1, nb, 2], F8, tag="poolW1")
        irT = persist.tile([F, nb], BF, tag="irT")
        avT = persist.tile([F, nb], BF, tag="avT")

        NCH = min(512, nb)
        with tc.tile_pool(name="pro_ps", bufs=2, space="PSUM") as pro_ps:
            # qh2 = [WiWq^T @ cand_even + c0 ; WiWq^T @ cand_odd + c0]
            qp = pro_ps.tile([2 * A, NPAIR], F32, tag="pps")
            nc.tensor.matmul(qp, wiq2a, candT_eo[:, 0:NPAIR],
                             start=True, stop=False)
            nc.tensor.matmul(qp, wiq2b, candT_eo[:, NPAIR:nb],
                             start=False, stop=True)
            nc.vector.tensor_scalar_add(qh2, qp, c0_2)

            # cand embedding (ascending order): cembT = Wi^T @ candT + bi
            for m in range(nb // NCH):
                cols = slice(m * NCH, (m + 1) * NCH)
                cep = pro_ps.tile([E, NCH], F32, tag="pps")
                nc.tensor.matmul(cep, wi, candT_asc[:, cols], start=True, stop=True)
                nc.scalar.activation(cembT[:, cols], cep, AF.Identity, bias=bi2)

        # ml pooling weights into poolW odd columns
        nc.vector.tensor_copy(poolW0[:, :, 1], mlT0)
        nc.vector.tensor_copy(poolW1[:, :, 1], mlT1)

        # ---------------- score + pooling pipeline (interleaved) ----------------
        pool_ps = ctx.enter_context(tc.tile_pool(name="pool_ps", bufs=2, space="PSUM"))
        score_ps = ctx.enter_context(contextlib.ExitStack())
        kh_ps = score_ps.enter_context(tc.tile_pool(name="kh_ps", bufs=3, space="PSUM"))
        sc_ps = score_ps.enter_context(tc.tile_pool(name="sc_ps", bufs=2, space="PSUM"))
        tr_ps = score_ps.enter_context(tc.tile_pool(name="tr_ps", bufs=1, space="PSUM"))
        at_ps = tr_ps

        def pool_group(g):
            pps = pool_ps.tile([F, 256], F32, tag="pps")
            for q in range(2):
                b0 = g * 128 + q * 64
                n0 = nat_p.tile([S0, 64, F], F8, tag="n0")
                nc.scalar.dma_start(out=n0, in_=d["nat"][0:S0, b0:b0 + 64, :])
                n1 = nat_p.tile([S1, 64, F], F8, tag="n1")
                nc.scalar.dma_start(out=n1, in_=d["nat"][S0:S, b0:b0 + 64, :])
                for i in range(64):
                    j = q * 64 + i
                    b = g * 128 + j
                    nc.tensor.matmul(pps[:, 2 * j:2 * j + 2], n0[:, i, :],
                                     poolW0[:, b, :], start=True, stop=False)
                    nc.tensor.matmul(pps[:, 2 * j:2 * j + 2], n1[:, i, :],
                                     poolW1[:, b, :], start=False, stop=True)
            v = pps.rearrange("p (b j) -> p b j", j=2)
            gs = slice(g * 128, (g + 1) * 128)
            nc.vector.tensor_copy(irT[:, gs], v[:, :, 0])
            nc.vector.tensor_copy(avT[:, gs], v[:, :, 1])

        for g in range(G):
            scT = sc_ps.tile([128, 512], F32, tag="scT")
            for q8 in range(4):
                b0 = g * 128 + q8 * 32
                ht = histT_p.tile([F, 32, S], F8, tag="ht")
                nc.sync.dma_start(out=ht, in_=d["histT"][:, b0:b0 + 32, :])
                for i in range(16):
                    pp = q8 * 16 + i         # pair within group
                    t = g * 64 + pp          # global pair
                    khp = kh_ps.tile([2 * A, S], F32, tag="khp")
                    nc.tensor.matmul(khp, wik2a, ht[:, 2 * i, :],
                                     start=True, stop=False)
                    nc.tensor.matmul(khp, wik2b, ht[:, 2 * i + 1, :],
                                     start=False, stop=True)
                    hid = hid_p.tile([2 * A, S], BF, tag="hid")
                    if t % 2 == 0:
                        nc.scalar.activation(hid, khp, AF.Relu,
                                             bias=qh2[:, t:t + 1])
                    else:
                        nc.vector.tensor_scalar(hid, khp, qh2[:, t:t + 1], 0.0,
                                                ALU.add, ALU.max)
                    # scores^T: [s, 2] per pair
                    nc.tensor.matmul(scT[0:S0, 2 * pp:2 * pp + 2],
                                     hid[:, 0:S0], wv2, start=True, stop=True)
                    nc.tensor.matmul(scT[0:S1, 128 + 2 * pp:130 + 2 * pp],
                                     hid[:, S0:S], wv2, start=True, stop=True)

            # drain scores^T, transpose to [b, s], masked softmax
            scT0 = grp_p.tile([S0, 128], F32, tag="scT0")
            scT1 = grp_p.tile([S1, 128], F32, tag="scT1")
            nc.vector.tensor_copy(scT0, scT[:, 0:128])
            nc.vector.tensor_copy(scT1, scT[0:S1, 128:256])
            scps = tr_ps.tile([128, S], F32, tag="trx")
            nc.tensor.transpose(scps[:, 0:S0], scT0, identf)
            nc.tensor.transpose(scps[:, S0:S], scT1, identf[0:S1, 0:S1])
            mtile = grp_p.tile([128, S], F32, tag="mtile")
            nc.sync.dma_start(out=mtile, in_=d["maskadd"][g * 128:(g + 1) * 128, :])
            scm = grp_p.tile([128, S], F32, tag="scm")
            nc.vector.tensor_add(scm, scps, mtile)
            esc = grp_p.tile([128, S], F32, tag="esc")
            nc.scalar.activation(esc, scm, AF.Exp)
            ssum = grp_p.tile([128, 1], F32, tag="ssum")
            nc.vector.tensor_reduce(ssum, esc, axis=mybir.AxisListType.X, op=ALU.add)
            rec = grp_p.tile([128, 1], F32, tag="rec")
            nc.vector.reciprocal(rec, ssum)
            attn = grp_p.tile([128, S], BF, tag="attn")
            nc.vector.tensor_scalar_mul(attn, esc, rec)
            # transpose attn back to [s, b] and write pool weights (even cols)
            atp = at_ps.tile([128, 256], BF, tag="trx")
            nc.tensor.transpose(atp[:, 0:128], attn[:, 0:S0], identb)
            nc.tensor.transpose(atp[0:S1, 128:256], attn[:, S0:S], identb)
            gs = slice(g * 128, (g + 1) * 128)
            nc.vector.tensor_copy(poolW0[:, gs, 0], atp[:, 0:128])
            nc.vector.tensor_copy(poolW1[:, gs, 0], atp[0:S1, 128:256])
            if g > 0:
                pool_group(g - 1)
        pool_group(G - 1)

        score_ps.close()

        # ---------------- embed + MLP head ----------------
        mlp_ps = ctx.enter_context(tc.tile_pool(name="mlp_ps", bufs=3, space="PSUM"))
        for m in range(nb // NCH):
            cols = slice(m * NCH, (m + 1) * NCH)
            ip = mlp_ps.tile([E, NCH], F32, tag="mp")
            nc.tensor.matmul(ip, wi, irT[:, cols], start=True, stop=True)
            eiT = mlp_p.tile([E, NCH], BF, tag="eiT")
            nc.scalar.activation(eiT, ip, AF.Identity, bias=bi2)
            ap_ = mlp_ps.tile([E, NCH], F32, tag="mp")
            nc.tensor.matmul(ap_, wi, avT[:, cols], start=True, stop=True)
            eaT = mlp_p.tile([E, NCH], BF, tag="eaT")
            nc.scalar.activation(eaT, ap_, AF.Identity, bias=bi2)

            h1p = mlp_ps.tile([128, NCH], F32, tag="mp")
            nc.tensor.matmul(h1p, w1a, eiT, start=True, stop=False)
            nc.tensor.matmul(h1p, w1b, cembT[:, cols], start=False, stop=False)
            nc.tensor.matmul(h1p, w1c, eaT, start=False, stop=True)
            h1 = mlp_p.tile([128, NCH], BF, tag="h1")
            nc.scalar.activation(h1, h1p, AF.Relu, bias=bn1h, scale=bn1s)

            h2p = mlp_ps.tile([64, NCH], F32, tag="mp")
            nc.tensor.matmul(h2p, w2, h1, start=True, stop=True)
            h2 = mlp_p.tile([64, NCH], BF, tag="h2")
            nc.scalar.activation(h2, h2p, AF.Relu, bias=bn2h, scale=bn2s)

            h3p = mlp_ps.tile([32, NCH], F32, tag="mp")
            nc.tensor.matmul(h3p, w3, h2, start=True, stop=True)
            h3 = mlp_p.tile([32, NCH], BF, tag="h3")
            nc.scalar.activation(h3, h3p, AF.Relu, bias=bn3h, scale=bn3s)

            lp = mlp_ps.tile([1, NCH], F32, tag="mp")
            nc.tensor.matmul(lp, wo, h3, start=True, stop=True)
            osb = mlp_p.tile([1, NCH], F32, tag="osb")
            nc.scalar.activation(osb, lp, AF.Sigmoid, bias=bo2[0:1, 0:1])
            nc.sync.dma_start(out=out_d[cols].unsqueeze(0), in_=osb)


_PROGRAMS = {}


def _build_program(nb):
    if nb in _PROGRAMS:
        return _PROGRAMS[nb]
    nc = bacc.Bacc("TRN2", target_bir_lowering=False, debug=False)
    d = _declare_inputs(nc, nb)
    out_d = nc.dram_tensor("out", [nb], F32, kind="ExternalOutput").ap()
    with tile.TileContext(nc) as tc:
        _body(tc, d, out_d, nb)
    nc.compile()
    _PROGRAMS[nb] = nc
    return nc


# ------------------------------------------------------------------ host ----

def _prep_maps(inputs, ncores=NCORES):
    nb = inputs["history_features"].shape[0] // ncores
    hist = np.asarray(inputs["history_features"], dtype=np.float32)
    cand = np.asarray(inputs["candidate_features"], dtype=np.float32)
    ln = np.asarray(inputs["history_length"]).astype(np.int64)
    b_tot = hist.shape[0]

    Wi = np.asarray(inputs["Wi"]); bi = np.asarray(inputs["bi"])
    Wq = np.asarray(inputs["Wq"]); Wk = np.asarray(inputs["Wk"])
    Wv = np.asarray(inputs["Wv"])

    hist_f8 = hist.astype(F8NP)
    histT_g = hist_f8.transpose(2, 0, 1)          # [F, B, S]
    nat_g = hist_f8.transpose(1, 0, 2)            # [S, B, F]
    candT = cand.astype(BF16NP).T                 # [F, B]

    seq = np.arange(S)
    mask = seq[None, :] < ln[:, None]
    maskadd_g = np.where(mask, np.float32(0), np.float32(-1e9)).astype(np.float32)
    mlT_g = np.where(mask, 1.0 / ln[:, None], 0.0).astype(F8NP).T  # [S, B]

    wik = (Wi @ Wk).astype(F8NP)
    wiq = (Wi @ Wq).astype(BF16NP)

    def stack2(w):
        wa = np.zeros((F, 128), dtype=w.dtype)
        wb = np.zeros((F, 128), dtype=w.dtype)
        wa[:, 0:A] = w
        wb[:, A:128] = w
        return wa, wb

    wik2a, wik2b = stack2(wik)
    wiq2a, wiq2b = stack2(wiq)
    c0 = (bi @ Wk + bi @ Wq).astype(np.float32)
    wv2 = np.zeros((2 * A, 2), dtype=BF16NP)
    wv2[0:A, 0] = Wv.astype(BF16NP)
    wv2[A:2 * A, 1] = Wv.astype(BF16NP)

    def bn_fold(b_, g_, be, m, v):
        s_ = (g_ / np.sqrt(v + EPS)).astype(np.float32)
        h_ = ((b_ - m) * s_ + be).astype(np.float32)
        return s_, h_

    s1, h1 = bn_fold(*(np.asarray(inputs[k]) for k in ("b1", "g1", "be1", "m1", "v1")))
    s2, h2 = bn_fold(*(np.asarray(inputs[k]) for k in ("b2", "g2", "be2", "m2", "v2")))
    s3, h3 = bn_fold(*(np.asarray(inputs[k]) for k in ("b3", "g3", "be3", "m3", "v3")))
    W1 = np.asarray(inputs["W1"])

    shared = {
        "wik2a": wik2a, "wik2b": wik2b, "wiq2a": wiq2a, "wiq2b": wiq2b,
        "wi": Wi.astype(BF16NP),
        "wv2": wv2,
        "c0_2": np.concatenate([c0, c0]).reshape(2 * A, 1),
        "bi2": bi.reshape(E, 1).astype(np.float32),
        "identf": np.eye(128, dtype=np.float32),
        "identb": np.eye(128, dtype=np.float32).astype(BF16NP),
        "w1a": W1[0:64].astype(BF16NP), "w1b": W1[64:128].astype(BF16NP),
        "w1c": W1[128:192].astype(BF16NP),
        "bn1s": s1.reshape(-1, 1), "bn1h": h1.reshape(-1, 1),
        "w2": np.asarray(inputs["W2"]).astype(BF16NP),
        "bn2s": s2.reshape(-1, 1), "bn2h": h2.reshape(-1, 1),
        "w3": np.asarray(inputs["W3"]).astype(BF16NP),
        "bn3s": s3.reshape(-1, 1), "bn3h": h3.reshape(-1, 1),
        "wo": np.asarray(inputs["Wo"]).astype(BF16NP),
        "bo2": np.asarray(inputs["bo"]).reshape(1, 1).astype(np.float32),
    }

    in_maps = []
    for c in range(ncores):
        cs = slice(c * nb, (c + 1) * nb)
        ct = np.ascontiguousarray(candT[:, cs])
        m = dict(shared)
        ht = np.empty((F, nb + 1, S), dtype=F8NP)
        ht[:, 0:nb, :] = histT_g[:, cs, :]
        m["histT"] = ht
        nt = np.empty((S, nb + 1, F), dtype=F8NP)
        nt[:, 0:nb, :] = nat_g[:, cs, :]
        m["nat"] = nt
        m["maskadd"] = np.ascontiguousarray(maskadd_g[cs])
        m["mlT"] = np.ascontiguousarray(mlT_g[:, cs])
        m["candT_eo"] = np.concatenate([ct[:, 0::2], ct[:, 1::2]], axis=1)
        m["candT_asc"] = ct
        in_maps.append(m)
    return nb, in_maps


LAST_RESULT = None


def kernel(**inputs):
    global LAST_RESULT
    import os
    nb, in_maps = _prep_maps(inputs)
    nc = _build_program(nb)
    trace = bool(int(os.environ.get("DIN_TRACE", "0")))
    res = bass_utils.run_bass_kernel_spmd(
        nc, in_maps, core_ids=list(range(NCORES)), trace=trace)
    LAST_RESULT = res
    out = np.concatenate([r["out"] for r in res.results])
    return out.astype(np.float32)
